# revision 19
# baseline (speedup 1.0000x reference)
"""ArcDecoder Bass kernel for 8 TRN2 NeuronCores.

Math (per node n, with norm_w/norm_b folded into weights host-side):
  zn   = LN(z)
  u1   = relu(zn @ W1eff + b1eff),  u2 = relu(zn @ W2eff + b2eff)
  h1n  = LN(u1), h2n = LN(u2)
  g    = h1n @ Wbil_eff
  score_e = dot(g[a0_e], h2n[a1_e]) + bil_b

Phase A (replicated): every core computes the full g/h2 node tables into its
own DRAM (bf16 matmuls, f32 LN stats).
Phase B (edges sharded E/8): per-edge rows fetched with the dma_gather custom
GPSIMD instruction (1024 rows per instruction, round-robin over 4 SWDGE
queues).  dma_gather takes int16 indices, so node ids >= 32768 gather from a
shifted table base; host groups each core's edges into 4 (head-half,
tail-half) buckets so every 1024-edge block is half-pure.  DVE multiply +
reduce forms the dots; host adds bil_b and inverse-permutes.
"""

import sys

if "/opt/trn_rl_repo" not in sys.path:
    sys.path.insert(0, "/opt/trn_rl_repo")

import numpy as np
import ml_dtypes

import concourse.bass as bass
import concourse.tile as tile
from concourse import bacc, mybir
from concourse.bass_utils import run_bass_kernel_spmd
from concourse.library_config import mlp
from concourse.masks import make_identity

N, D, E = 50000, 128, 500000
NCORES = 8
P = 128
SHARD_T = 49                     # node tiles per core (sharded phase A)
SHARD = SHARD_T * P              # 6272 rows per core
NT = SHARD_T * NCORES            # 392 node tiles total
NPAD = NT * P                    # 50176
TB = 4                           # node subtiles per z-load / table-store batch
EC = E // NCORES                 # 62500 edges per core
HALF = 32768                     # int16 index ceiling for dma_gather

F32 = mybir.dt.float32
BF16 = mybir.dt.bfloat16
AF = mybir.ActivationFunctionType
ALU = mybir.AluOpType

TABLE_DT = BF16                  # dtype of g/h2 tables (gather payload)
MM_DT = BF16                     # dtype of matmul operands in phase A
BLK = 1024                       # edges per gather block
CB = BLK // P                    # row-chunks per partition in a gather tile
NQ = 4                           # SWDGE queues
EPS = 1e-5

_np_tdt = np.float32 if TABLE_DT == F32 else ml_dtypes.bfloat16
_np_mdt = np.float32 if MM_DT == F32 else ml_dtypes.bfloat16


def _build(block_specs):
    """block_specs: list of (head_half, tail_half) per 1024-edge block."""
    nblk = len(block_specs)
    ecpad = nblk * BLK
    S = BLK // 16

    nc = bacc.Bacc("TRN2", target_bir_lowering=False, debug=False,
                   num_devices=NCORES, num_swdge_queues=NQ)

    z_ext = nc.dram_tensor("z", [SHARD, D], F32, kind="ExternalInput").ap()
    wcat_ext = nc.dram_tensor("wcat", [D, 2 * D], MM_DT, kind="ExternalInput").ap()
    bcat_ext = nc.dram_tensor("bcat", [1, 2 * D], MM_DT, kind="ExternalInput").ap()
    wbil_ext = nc.dram_tensor("wbil", [D, D], MM_DT, kind="ExternalInput").ap()
    iotac_ext = nc.dram_tensor("iotac", [P, 1], F32, kind="ExternalInput").ap()
    idx0_ext = nc.dram_tensor("idx0", [nblk, P, S], mybir.dt.int16,
                              kind="ExternalInput").ap()
    lidxr_ext = nc.dram_tensor("lidxr", [nblk, 8, P], MM_DT,
                               kind="ExternalInput").ap()
    out_ext = nc.dram_tensor("out", [ecpad], F32, kind="ExternalOutput").ap()

    g_shard = nc.dram_tensor("g_shard", [SHARD, D], TABLE_DT).ap()
    g_table = nc.dram_tensor("g_table", [NPAD, D], TABLE_DT, addr_space="Shared").ap()

    with tile.TileContext(nc) as tc:
        with (
            tc.tile_pool(name="const", bufs=1) as const_p,
            tc.tile_pool(name="zload", bufs=2) as zload_p,
            tc.tile_pool(name="work", bufs=6) as work_p,
            tc.tile_pool(name="stat", bufs=8) as stat_p,
            tc.tile_pool(name="tabout", bufs=2) as tabout_p,
            tc.tile_pool(name="psum", bufs=2, space="PSUM") as psum_p,
            tc.tile_pool(name="gather", bufs=12) as gather_p,
            tc.tile_pool(name="idx", bufs=12) as idx_p,
            tc.tile_pool(name="score", bufs=4) as score_p,
        ):
            # ---- constants -------------------------------------------------
            nc.gpsimd.load_library(mlp)
            ident = const_p.tile([P, P], MM_DT)
            make_identity(nc, ident[:])
            wcat_sb = const_p.tile([D, 2 * D], MM_DT)
            nc.sync.dma_start(wcat_sb[:], wcat_ext[:])
            bcat_sb = const_p.tile([1, 2 * D], MM_DT)
            nc.sync.dma_start(bcat_sb[:], bcat_ext[:])
            wbil_sb = const_p.tile([D, D], MM_DT)
            nc.sync.dma_start(wbil_sb[:], wbil_ext[:])
            ones_row = const_p.tile([1, P], MM_DT)
            nc.vector.memset(ones_row[:], 1.0)
            epsc = const_p.tile([P, 1], F32)
            nc.vector.memset(epsc[:], EPS)
            h_sb = const_p.tile([P, SHARD_T, D], TABLE_DT)
            iotac_ext2 = iotac_ext  # loaded below
            iotac = const_p.tile([P, 1], F32)
            nc.sync.dma_start(iotac[:], iotac_ext[:])

            # ---- phase A: node tables (sharded; each core its z-shard) ----
            for b0 in range(0, SHARD_T, TB):
                tb = min(TB, SHARD_T - b0)
                n0 = b0 * P
                zbatch = zload_p.tile([P, TB, D], F32, tag="zbatch")
                nc.sync.dma_start(
                    zbatch[:, :tb, :],
                    z_ext[n0:n0 + tb * P, :].rearrange("(j p) d -> p j d", p=P),
                )
                gbuf = tabout_p.tile([P, TB, D], TABLE_DT, tag="gbuf")

                for jj in range(tb):
                    z_t = zbatch[:, jj, :]
                    st = stat_p.tile([P, 6], F32, tag="st")
                    nc.vector.bn_stats(st[:], z_t)
                    mv = stat_p.tile([P, 2], F32, tag="mv")
                    nc.vector.bn_aggr(mv[:], st[:])
                    sd = stat_p.tile([P, 1], F32, tag="sd")
                    nc.scalar.activation(sd[:], mv[:, 1:2], AF.Sqrt, bias=epsc[:])
                    ri = stat_p.tile([P, 1], F32, tag="ri")
                    nc.vector.reciprocal(ri[:], sd[:])
                    zn = work_p.tile([P, D], MM_DT, tag="zn")
                    nc.vector.tensor_scalar(zn[:], z_t, mv[:, 0:1], ri[:],
                                            ALU.subtract, ALU.mult)
                    znT_ps = psum_p.tile([P, P], MM_DT, tag="tpos", space="PSUM")
                    nc.tensor.transpose(znT_ps[:], zn[:], ident[:])
                    znT = work_p.tile([P, P], MM_DT, tag="znT")
                    nc.scalar.copy(znT[:], znT_ps[:])
                    u12_ps = psum_p.tile([P, 2 * D], F32, tag="mm12", space="PSUM")
                    nc.tensor.matmul(u12_ps[:], lhsT=znT[:], rhs=wcat_sb[:],
                                     start=True, stop=False)
                    nc.tensor.matmul(u12_ps[:], lhsT=ones_row[:], rhs=bcat_sb[:],
                                     start=False, stop=True)
                    u12 = work_p.tile([P, 2 * D], F32, tag="u12")
                    nc.scalar.activation(u12[:], u12_ps[:], AF.Relu)
                    st1 = stat_p.tile([P, 6], F32, tag="st1")
                    nc.vector.bn_stats(st1[:], u12[:, 0:D])
                    mv1 = stat_p.tile([P, 2], F32, tag="mv1")
                    nc.vector.bn_aggr(mv1[:], st1[:])
                    sd1 = stat_p.tile([P, 1], F32, tag="sd1")
                    nc.scalar.activation(sd1[:], mv1[:, 1:2], AF.Sqrt, bias=epsc[:])
                    ri1 = stat_p.tile([P, 1], F32, tag="ri1")
                    nc.vector.reciprocal(ri1[:], sd1[:])
                    st2 = stat_p.tile([P, 6], F32, tag="st2")
                    nc.vector.bn_stats(st2[:], u12[:, D:2 * D])
                    mv2 = stat_p.tile([P, 2], F32, tag="mv2")
                    nc.vector.bn_aggr(mv2[:], st2[:])
                    sd2 = stat_p.tile([P, 1], F32, tag="sd2")
                    nc.scalar.activation(sd2[:], mv2[:, 1:2], AF.Sqrt, bias=epsc[:])
                    ri2 = stat_p.tile([P, 1], F32, tag="ri2")
                    nc.vector.reciprocal(ri2[:], sd2[:])
                    h1n = work_p.tile([P, D], MM_DT, tag="h1n")
                    nc.vector.tensor_scalar(h1n[:], u12[:, 0:D], mv1[:, 0:1],
                                            ri1[:], ALU.subtract, ALU.mult)
                    nc.vector.tensor_scalar(h_sb[:, b0 + jj, :], u12[:, D:2 * D],
                                            mv2[:, 0:1], ri2[:],
                                            ALU.subtract, ALU.mult)
                    h1nT_ps = psum_p.tile([P, P], MM_DT, tag="tpos", space="PSUM")
                    nc.tensor.transpose(h1nT_ps[:], h1n[:], ident[:])
                    h1nT = work_p.tile([P, P], MM_DT, tag="h1nT")
                    nc.scalar.copy(h1nT[:], h1nT_ps[:])
                    g_ps = psum_p.tile([P, D], F32, tag="gmm", space="PSUM")
                    nc.tensor.matmul(g_ps[:], lhsT=h1nT[:], rhs=wbil_sb[:],
                                     start=True, stop=True)
                    nc.scalar.copy(gbuf[:, jj, :], g_ps[:])

                nc.sync.dma_start(
                    g_shard[n0:n0 + tb * P, :].rearrange("(j p) d -> p j d", p=P),
                    gbuf[:, :tb, :],
                )


            # ---- all-gather shards into full tables -----------------------
            nc.gpsimd.collective_compute(
                "AllGather", ALU.bypass,
                replica_groups=[list(range(NCORES))],
                ins=[g_shard[:]], outs=[g_table[:]],
            )


            # ---- phase B: head dma_gather + tail selection matmul ---------
            for b, (h0, chunks) in enumerate(block_specs):
                i0 = idx_p.tile([P, S], mybir.dt.int16, tag="i0")
                nc.sync.dma_start(i0[:], idx0_ext[b])
                lxr = idx_p.tile([1, CB * P], MM_DT, tag="lxr")
                nc.sync.dma_start(lxr[:], lidxr_ext[b].rearrange("j p -> (j p)")[None, :])
                g_src = g_table[HALF:, :] if h0 else g_table[:, :]
                gt = gather_p.tile([P, CB, D], TABLE_DT, tag="gt")
                nc.gpsimd.dma_gather(gt[:], g_src, i0[:], BLK, BLK, D,
                                     queue_num=b % NQ)
                ht = gather_p.tile([P, CB, D], TABLE_DT, tag="ht")
                for j, ch in enumerate(chunks):
                    # broadcast lidx row across partitions via K=1 matmul
                    lb_ps = psum_p.tile([P, P], F32, tag="lbps", space="PSUM")
                    nc.tensor.matmul(lb_ps[:], lhsT=ones_row[:],
                                     rhs=lxr[0:1, j * P:(j + 1) * P], start=True, stop=True)
                    sT = work_p.tile([P, P], MM_DT, tag="sT")
                    nc.vector.tensor_scalar(sT[:], lb_ps[:], iotac[:], None,
                                            ALU.is_equal)
                    sel_ps = psum_p.tile([P, D], F32, tag="selps", space="PSUM")
                    nc.tensor.matmul(sel_ps[:], lhsT=sT[:],
                                     rhs=h_sb[:, ch, :], start=True, stop=True)
                    nc.scalar.copy(ht[:, j, :], sel_ps[:])
                nc.vector.tensor_tensor(gt[:], gt[:], ht[:], op=ALU.mult)
                sc = score_p.tile([P, CB], F32, tag="sc")
                nc.vector.tensor_reduce(
                    sc[:], gt[:], axis=mybir.AxisListType.X, op=ALU.add,
                )
                # edge k of block b sits at [k % 128, k // 128]
                nc.sync.dma_start(
                    out_ext[b * BLK:(b + 1) * BLK].rearrange("(j p) -> p j", p=P),
                    sc[:],
                )

    nc.compile()
    return nc


_CACHE = {}
_RUN_KWARGS = {}
LAST_RESULTS = None


def _pack_idx(vals):
    """[nblk, 1024] int16 -> dma_gather SBUF layout [nblk, 128, 64]:
    index k lives at partition k%16, column k//16, replicated into all
    eight 16-partition groups."""
    nblk = vals.shape[0]
    w = vals.reshape(nblk, BLK // 16, 16).transpose(0, 2, 1)   # [nblk,16,S]
    return np.tile(w, (1, 8, 1)).astype(np.int16)


def kernel(**inputs) -> np.ndarray:
    z = np.asarray(inputs["z"], np.float32)
    pot_arcs = np.asarray(inputs["pot_arcs"])
    lin1_w = np.asarray(inputs["lin1_w"], np.float32)
    lin1_b = np.asarray(inputs["lin1_b"], np.float32)
    lin2_w = np.asarray(inputs["lin2_w"], np.float32)
    lin2_b = np.asarray(inputs["lin2_b"], np.float32)
    bil_w = np.asarray(inputs["bil_w"], np.float32)
    bil_b = np.asarray(inputs["bil_b"], np.float32)
    norm_w = np.asarray(inputs["norm_w"], np.float32)
    norm_b = np.asarray(inputs["norm_b"], np.float32)

    if not np.allclose(norm_b, 0.0):
        # general norm_b adds per-node scalar terms; not exercised by this
        # problem's inputs.  Exact numpy fallback keeps kernel() total.
        return _numpy_reference(z, pot_arcs, lin1_w, lin1_b, lin2_w, lin2_b,
                                bil_w, bil_b, norm_w, norm_b)

    w1eff = norm_w[:, None] * lin1_w.T
    b1eff = norm_b @ lin1_w.T + lin1_b
    w2eff = norm_w[:, None] * lin2_w.T
    b2eff = norm_b @ lin2_w.T + lin2_b
    wbil = bil_w[0] * norm_w[None, :]
    wcat = np.concatenate([w1eff, w2eff], axis=1).astype(_np_mdt)
    bcat = np.concatenate([b1eff, b2eff])[None, :].astype(_np_mdt)
    wbil = wbil.astype(_np_mdt)

    zpad = np.zeros((NPAD, D), np.float32)
    zpad[:N] = z

    a0 = pot_arcs[:, 0].astype(np.int32)
    a1 = pot_arcs[:, 1].astype(np.int32)

    # --- assign edges to tail-owner cores; tile by (head-half, tail-chunk) -
    core_of_edge = a1 // SHARD
    per_core = []
    for c in range(NCORES):
        eids = np.where(core_of_edge == c)[0]
        a0c = a0[eids]
        l1 = a1[eids] - c * SHARD
        bucket = (a0c >= HALF).astype(np.int64)
        chunk = l1 // P
        # sort by (bucket, chunk) and cut chunk-pure 128-edge tiles
        order = np.lexsort((chunk, bucket))
        per_core.append((eids, a0c, l1, bucket, chunk, order))

    # tiles per (bucket, chunk) must be uniform across cores (same graph)
    ntile_bc = np.zeros((2, SHARD_T), np.int64)
    for c in range(NCORES):
        _, _, _, bucket, chunk, _ = per_core[c]
        for bkt in range(2):
            cnt = np.bincount(chunk[bucket == bkt], minlength=SHARD_T)
            ntile_bc[bkt] = np.maximum(ntile_bc[bkt], -(-cnt // P))
    # tile list: (bucket, chunk) repeated; pad each bucket to multiple of 8
    tiles = []
    for bkt in range(2):
        start = len(tiles)
        for ch in range(SHARD_T):
            tiles += [(bkt, ch)] * int(ntile_bc[bkt, ch])
        while (len(tiles) - start) % CB:
            tiles.append((bkt, 0))
    ntiles = len(tiles)
    nblk = ntiles // CB
    ecpad = ntiles * P

    block_specs = []
    for b in range(nblk):
        bts = tiles[b * CB:(b + 1) * CB]
        assert len({t[0] for t in bts}) == 1
        block_specs.append((bts[0][0], tuple(t[1] for t in bts)))

    # slot ranges per (bucket, chunk)
    tile_start = {}
    pos = 0
    for t in tiles:
        tile_start.setdefault(t, []).append(pos)
        pos += P

    in_maps = []
    gathers = []
    iota_col = np.arange(P, dtype=np.float32).reshape(P, 1)
    for c in range(NCORES):
        eids, a0c, l1, bucket, chunk, order = per_core[c]
        i0 = np.zeros(ecpad, np.int32)
        lidx = np.zeros(ecpad, np.int32)
        gid = np.empty(len(eids), np.int64)
        for bkt in range(2):
            for ch in range(SHARD_T):
                sel = order[(bucket[order] == bkt) & (chunk[order] == ch)]
                starts = tile_start[(bkt, ch)]
                for ti in range(len(starts)):
                    seg = sel[ti * P:(ti + 1) * P]
                    dst = starts[ti] + np.arange(len(seg))
                    i0[dst] = a0c[seg] - (HALF if bkt else 0)
                    lidx[dst] = l1[seg] - ch * P
                    gid[seg] = dst
        gathers.append((eids, gid))
        # lidxr[b, j, p] = lidx of edge b*1024 + j*128 + p
        lidxr = lidx.astype(np.float32).reshape(nblk, CB, P)
        in_maps.append({
            "z": zpad[c * SHARD:(c + 1) * SHARD],
            "wcat": wcat,
            "bcat": bcat,
            "wbil": wbil,
            "iotac": iota_col,
            "idx0": _pack_idx(i0.astype(np.int16).reshape(nblk, BLK)),
            "lidxr": lidxr.astype(_np_mdt),
        })

    key = tuple(block_specs)
    if key not in _CACHE:
        _CACHE[key] = _build(block_specs)
    nc = _CACHE[key]

    res = run_bass_kernel_spmd(nc, in_maps, list(range(NCORES)), **_RUN_KWARGS)
    global LAST_RESULTS
    LAST_RESULTS = res

    scores = np.empty(E, np.float32)
    for c in range(NCORES):
        out_c = np.asarray(res.results[c]["out"], np.float32)
        eids, gid = gathers[c]
        scores[eids] = out_c[gid]
    return scores + float(bil_b[0])


def _numpy_reference(z, pot_arcs, lin1_w, lin1_b, lin2_w, lin2_b,
                     bil_w, bil_b, norm_w, norm_b):
    def ln(x):
        mu = x.mean(-1, keepdims=True)
        var = x.var(-1, keepdims=True)
        return (x - mu) / np.sqrt(var + 1e-5) * norm_w + norm_b

    zn = ln(z)
    h1 = ln(np.maximum(zn @ lin1_w.T + lin1_b, 0.0))
    h2 = ln(np.maximum(zn @ lin2_w.T + lin2_b, 0.0))
    g = h1 @ bil_w[0]
    a0 = pot_arcs[:, 0].astype(np.int64)
    a1 = pot_arcs[:, 1].astype(np.int64)
    return np.einsum("ed,ed->e", g[a0], h2[a1]) + bil_b[0]


# revision 20
# speedup vs baseline: 1.0272x; 1.0272x over previous
"""ArcDecoder Bass kernel for 8 TRN2 NeuronCores.

Math (per node n, with norm_w/norm_b folded into weights host-side):
  zn   = LN(z)
  u1   = relu(zn @ W1eff + b1eff),  u2 = relu(zn @ W2eff + b2eff)
  h1n  = LN(u1), h2n = LN(u2)
  g    = h1n @ Wbil_eff
  score_e = dot(g[a0_e], h2n[a1_e]) + bil_b

Phase A (replicated): every core computes the full g/h2 node tables into its
own DRAM (bf16 matmuls, f32 LN stats).
Phase B (edges sharded E/8): per-edge rows fetched with the dma_gather custom
GPSIMD instruction (1024 rows per instruction, round-robin over 4 SWDGE
queues).  dma_gather takes int16 indices, so node ids >= 32768 gather from a
shifted table base; host groups each core's edges into 4 (head-half,
tail-half) buckets so every 1024-edge block is half-pure.  DVE multiply +
reduce forms the dots; host adds bil_b and inverse-permutes.
"""

import sys

if "/opt/trn_rl_repo" not in sys.path:
    sys.path.insert(0, "/opt/trn_rl_repo")

import numpy as np
import ml_dtypes

import concourse.bass as bass
import concourse.tile as tile
from concourse import bacc, mybir
from concourse.bass_utils import run_bass_kernel_spmd
from concourse.library_config import mlp
from concourse.masks import make_identity

N, D, E = 50000, 128, 500000
NCORES = 8
P = 128
SHARD_T = 49                     # node tiles per core (sharded phase A)
SHARD = SHARD_T * P              # 6272 rows per core
NT = SHARD_T * NCORES            # 392 node tiles total
NPAD = NT * P                    # 50176
TB = 4                           # node subtiles per z-load / table-store batch
EC = E // NCORES                 # 62500 edges per core
HALF = 32768                     # int16 index ceiling for dma_gather

F32 = mybir.dt.float32
BF16 = mybir.dt.bfloat16
AF = mybir.ActivationFunctionType
ALU = mybir.AluOpType

TABLE_DT = BF16                  # dtype of g/h2 tables (gather payload)
MM_DT = BF16                     # dtype of matmul operands in phase A
BLK = 1024                       # edges per gather block
CB = BLK // P                    # row-chunks per partition in a gather tile
NQ = 4                           # SWDGE queues
EPS = 1e-5

_np_tdt = np.float32 if TABLE_DT == F32 else ml_dtypes.bfloat16
_np_mdt = np.float32 if MM_DT == F32 else ml_dtypes.bfloat16


def _build(block_specs):
    """block_specs: list of (head_half, tail_half) per 1024-edge block."""
    nblk = len(block_specs)
    ecpad = nblk * BLK
    S = BLK // 16

    nc = bacc.Bacc("TRN2", target_bir_lowering=False, debug=False,
                   num_devices=NCORES, num_swdge_queues=NQ)

    z_ext = nc.dram_tensor("z", [SHARD, D], F32, kind="ExternalInput").ap()
    wcat_ext = nc.dram_tensor("wcat", [D, 2 * D], MM_DT, kind="ExternalInput").ap()
    bcat_ext = nc.dram_tensor("bcat", [1, 2 * D], MM_DT, kind="ExternalInput").ap()
    wbil_ext = nc.dram_tensor("wbil", [D, D], MM_DT, kind="ExternalInput").ap()
    iotac_ext = nc.dram_tensor("iotac", [P, 1], F32, kind="ExternalInput").ap()
    idx0_ext = nc.dram_tensor("idx0", [nblk, P, S], mybir.dt.int16,
                              kind="ExternalInput").ap()
    lidxr_ext = nc.dram_tensor("lidxr", [nblk, 8, P], MM_DT,
                               kind="ExternalInput").ap()
    out_ext = nc.dram_tensor("out", [ecpad], F32, kind="ExternalOutput").ap()

    g_shard = nc.dram_tensor("g_shard", [SHARD, D], TABLE_DT).ap()
    g_table = nc.dram_tensor("g_table", [NPAD, D], TABLE_DT, addr_space="Shared").ap()

    with tile.TileContext(nc) as tc:
        with (
            tc.tile_pool(name="const", bufs=1) as const_p,
            tc.tile_pool(name="zload", bufs=2) as zload_p,
            tc.tile_pool(name="work", bufs=6) as work_p,
            tc.tile_pool(name="stat", bufs=8) as stat_p,
            tc.tile_pool(name="tabout", bufs=2) as tabout_p,
            tc.tile_pool(name="gather", bufs=12) as gather_p,
            tc.tile_pool(name="idx", bufs=12) as idx_p,
            tc.tile_pool(name="score", bufs=4) as score_p,
        ):
            # ---- constants -------------------------------------------------
            nc.gpsimd.load_library(mlp)
            ident = const_p.tile([P, P], MM_DT)
            make_identity(nc, ident[:])
            wcat_sb = const_p.tile([D, 2 * D], MM_DT)
            nc.sync.dma_start(wcat_sb[:], wcat_ext[:])
            bcat_sb = const_p.tile([1, 2 * D], MM_DT)
            nc.sync.dma_start(bcat_sb[:], bcat_ext[:])
            wbil_sb = const_p.tile([D, D], MM_DT)
            nc.sync.dma_start(wbil_sb[:], wbil_ext[:])
            ones_row = const_p.tile([1, P], MM_DT)
            nc.vector.memset(ones_row[:], 1.0)
            epsc = const_p.tile([P, 1], F32)
            nc.vector.memset(epsc[:], EPS)
            h_sb = const_p.tile([P, SHARD_T, D], TABLE_DT)
            iotac_ext2 = iotac_ext  # loaded below
            iotac = const_p.tile([P, 1], F32)
            nc.sync.dma_start(iotac[:], iotac_ext[:])

            # ---- phase A: node tables (sharded; each core its z-shard) ----
            pa = tc.tile_pool(name="psumA", bufs=2, space="PSUM")
            psum_p = pa.__enter__()
            for b0 in range(0, SHARD_T, TB):
                tb = min(TB, SHARD_T - b0)
                n0 = b0 * P
                zbatch = zload_p.tile([P, TB, D], F32, tag="zbatch")
                nc.sync.dma_start(
                    zbatch[:, :tb, :],
                    z_ext[n0:n0 + tb * P, :].rearrange("(j p) d -> p j d", p=P),
                )
                gbuf = tabout_p.tile([P, TB, D], TABLE_DT, tag="gbuf")

                for jj in range(tb):
                    z_t = zbatch[:, jj, :]
                    st = stat_p.tile([P, 6], F32, tag="st")
                    nc.vector.bn_stats(st[:], z_t)
                    mv = stat_p.tile([P, 2], F32, tag="mv")
                    nc.vector.bn_aggr(mv[:], st[:])
                    sd = stat_p.tile([P, 1], F32, tag="sd")
                    nc.scalar.activation(sd[:], mv[:, 1:2], AF.Sqrt, bias=epsc[:])
                    ri = stat_p.tile([P, 1], F32, tag="ri")
                    nc.vector.reciprocal(ri[:], sd[:])
                    zn = work_p.tile([P, D], MM_DT, tag="zn")
                    nc.vector.tensor_scalar(zn[:], z_t, mv[:, 0:1], ri[:],
                                            ALU.subtract, ALU.mult)
                    znT_ps = psum_p.tile([P, P], MM_DT, tag="tpos", space="PSUM")
                    nc.tensor.transpose(znT_ps[:], zn[:], ident[:])
                    znT = work_p.tile([P, P], MM_DT, tag="znT")
                    nc.scalar.copy(znT[:], znT_ps[:])
                    u12_ps = psum_p.tile([P, 2 * D], F32, tag="mm12", space="PSUM")
                    nc.tensor.matmul(u12_ps[:], lhsT=znT[:], rhs=wcat_sb[:],
                                     start=True, stop=False)
                    nc.tensor.matmul(u12_ps[:], lhsT=ones_row[:], rhs=bcat_sb[:],
                                     start=False, stop=True)
                    u12 = work_p.tile([P, 2 * D], F32, tag="u12")
                    nc.scalar.activation(u12[:], u12_ps[:], AF.Relu)
                    st1 = stat_p.tile([P, 6], F32, tag="st1")
                    nc.vector.bn_stats(st1[:], u12[:, 0:D])
                    mv1 = stat_p.tile([P, 2], F32, tag="mv1")
                    nc.vector.bn_aggr(mv1[:], st1[:])
                    sd1 = stat_p.tile([P, 1], F32, tag="sd1")
                    nc.scalar.activation(sd1[:], mv1[:, 1:2], AF.Sqrt, bias=epsc[:])
                    ri1 = stat_p.tile([P, 1], F32, tag="ri1")
                    nc.vector.reciprocal(ri1[:], sd1[:])
                    st2 = stat_p.tile([P, 6], F32, tag="st2")
                    nc.vector.bn_stats(st2[:], u12[:, D:2 * D])
                    mv2 = stat_p.tile([P, 2], F32, tag="mv2")
                    nc.vector.bn_aggr(mv2[:], st2[:])
                    sd2 = stat_p.tile([P, 1], F32, tag="sd2")
                    nc.scalar.activation(sd2[:], mv2[:, 1:2], AF.Sqrt, bias=epsc[:])
                    ri2 = stat_p.tile([P, 1], F32, tag="ri2")
                    nc.vector.reciprocal(ri2[:], sd2[:])
                    h1n = work_p.tile([P, D], MM_DT, tag="h1n")
                    nc.vector.tensor_scalar(h1n[:], u12[:, 0:D], mv1[:, 0:1],
                                            ri1[:], ALU.subtract, ALU.mult)
                    nc.vector.tensor_scalar(h_sb[:, b0 + jj, :], u12[:, D:2 * D],
                                            mv2[:, 0:1], ri2[:],
                                            ALU.subtract, ALU.mult)
                    h1nT_ps = psum_p.tile([P, P], MM_DT, tag="tpos", space="PSUM")
                    nc.tensor.transpose(h1nT_ps[:], h1n[:], ident[:])
                    h1nT = work_p.tile([P, P], MM_DT, tag="h1nT")
                    nc.scalar.copy(h1nT[:], h1nT_ps[:])
                    g_ps = psum_p.tile([P, D], F32, tag="gmm", space="PSUM")
                    nc.tensor.matmul(g_ps[:], lhsT=h1nT[:], rhs=wbil_sb[:],
                                     start=True, stop=True)
                    nc.scalar.copy(gbuf[:, jj, :], g_ps[:])

                nc.sync.dma_start(
                    g_shard[n0:n0 + tb * P, :].rearrange("(j p) d -> p j d", p=P),
                    gbuf[:, :tb, :],
                )


            # ---- all-gather shards into full tables -----------------------
            nc.gpsimd.collective_compute(
                "AllGather", ALU.bypass,
                replica_groups=[list(range(NCORES))],
                ins=[g_shard[:]], outs=[g_table[:]],
            )
            pa.__exit__(None, None, None)
            pb = tc.tile_pool(name="psumB", bufs=4, space="PSUM")
            psum_p = pb.__enter__()


            # ---- phase B: head dma_gather + tail selection matmul ---------
            for b, (h0, chunks) in enumerate(block_specs):
                i0 = idx_p.tile([P, S], mybir.dt.int16, tag="i0")
                nc.sync.dma_start(i0[:], idx0_ext[b])
                lxr = idx_p.tile([1, CB * P], MM_DT, tag="lxr")
                nc.sync.dma_start(lxr[:], lidxr_ext[b].rearrange("j p -> (j p)")[None, :])
                g_src = g_table[HALF:, :] if h0 else g_table[:, :]
                gt = gather_p.tile([P, CB, D], TABLE_DT, tag="gt")
                nc.gpsimd.dma_gather(gt[:], g_src, i0[:], BLK, BLK, D,
                                     queue_num=b % NQ)
                ht = gather_p.tile([P, CB, D], TABLE_DT, tag="ht")
                for j, ch in enumerate(chunks):
                    # broadcast lidx row across partitions via K=1 matmul
                    lb_ps = psum_p.tile([P, P], F32, tag="lbps", space="PSUM")
                    nc.tensor.matmul(lb_ps[:], lhsT=ones_row[:],
                                     rhs=lxr[0:1, j * P:(j + 1) * P], start=True, stop=True)
                    sT = work_p.tile([P, P], MM_DT, tag="sT")
                    nc.vector.tensor_scalar(sT[:], lb_ps[:], iotac[:], None,
                                            ALU.is_equal)
                    sel_ps = psum_p.tile([P, D], F32, tag="selps", space="PSUM")
                    nc.tensor.matmul(sel_ps[:], lhsT=sT[:],
                                     rhs=h_sb[:, ch, :], start=True, stop=True)
                    nc.scalar.copy(ht[:, j, :], sel_ps[:])
                nc.vector.tensor_tensor(gt[:], gt[:], ht[:], op=ALU.mult)
                sc = score_p.tile([P, CB], F32, tag="sc")
                nc.vector.tensor_reduce(
                    sc[:], gt[:], axis=mybir.AxisListType.X, op=ALU.add,
                )
                # edge k of block b sits at [k % 128, k // 128]
                nc.sync.dma_start(
                    out_ext[b * BLK:(b + 1) * BLK].rearrange("(j p) -> p j", p=P),
                    sc[:],
                )

            pb.__exit__(None, None, None)

    nc.compile()
    return nc


_CACHE = {}
_RUN_KWARGS = {}
LAST_RESULTS = None


def _pack_idx(vals):
    """[nblk, 1024] int16 -> dma_gather SBUF layout [nblk, 128, 64]:
    index k lives at partition k%16, column k//16, replicated into all
    eight 16-partition groups."""
    nblk = vals.shape[0]
    w = vals.reshape(nblk, BLK // 16, 16).transpose(0, 2, 1)   # [nblk,16,S]
    return np.tile(w, (1, 8, 1)).astype(np.int16)


def kernel(**inputs) -> np.ndarray:
    z = np.asarray(inputs["z"], np.float32)
    pot_arcs = np.asarray(inputs["pot_arcs"])
    lin1_w = np.asarray(inputs["lin1_w"], np.float32)
    lin1_b = np.asarray(inputs["lin1_b"], np.float32)
    lin2_w = np.asarray(inputs["lin2_w"], np.float32)
    lin2_b = np.asarray(inputs["lin2_b"], np.float32)
    bil_w = np.asarray(inputs["bil_w"], np.float32)
    bil_b = np.asarray(inputs["bil_b"], np.float32)
    norm_w = np.asarray(inputs["norm_w"], np.float32)
    norm_b = np.asarray(inputs["norm_b"], np.float32)

    if not np.allclose(norm_b, 0.0):
        # general norm_b adds per-node scalar terms; not exercised by this
        # problem's inputs.  Exact numpy fallback keeps kernel() total.
        return _numpy_reference(z, pot_arcs, lin1_w, lin1_b, lin2_w, lin2_b,
                                bil_w, bil_b, norm_w, norm_b)

    w1eff = norm_w[:, None] * lin1_w.T
    b1eff = norm_b @ lin1_w.T + lin1_b
    w2eff = norm_w[:, None] * lin2_w.T
    b2eff = norm_b @ lin2_w.T + lin2_b
    wbil = bil_w[0] * norm_w[None, :]
    wcat = np.concatenate([w1eff, w2eff], axis=1).astype(_np_mdt)
    bcat = np.concatenate([b1eff, b2eff])[None, :].astype(_np_mdt)
    wbil = wbil.astype(_np_mdt)

    zpad = np.zeros((NPAD, D), np.float32)
    zpad[:N] = z

    a0 = pot_arcs[:, 0].astype(np.int32)
    a1 = pot_arcs[:, 1].astype(np.int32)

    # --- assign edges to tail-owner cores; tile by (head-half, tail-chunk) -
    core_of_edge = a1 // SHARD
    per_core = []
    for c in range(NCORES):
        eids = np.where(core_of_edge == c)[0]
        a0c = a0[eids]
        l1 = a1[eids] - c * SHARD
        bucket = (a0c >= HALF).astype(np.int64)
        chunk = l1 // P
        # sort by (bucket, chunk) and cut chunk-pure 128-edge tiles
        order = np.lexsort((chunk, bucket))
        per_core.append((eids, a0c, l1, bucket, chunk, order))

    # tiles per (bucket, chunk) must be uniform across cores (same graph)
    ntile_bc = np.zeros((2, SHARD_T), np.int64)
    for c in range(NCORES):
        _, _, _, bucket, chunk, _ = per_core[c]
        for bkt in range(2):
            cnt = np.bincount(chunk[bucket == bkt], minlength=SHARD_T)
            ntile_bc[bkt] = np.maximum(ntile_bc[bkt], -(-cnt // P))
    # tile list: (bucket, chunk) repeated; pad each bucket to multiple of 8
    tiles = []
    for bkt in range(2):
        start = len(tiles)
        for ch in range(SHARD_T):
            tiles += [(bkt, ch)] * int(ntile_bc[bkt, ch])
        while (len(tiles) - start) % CB:
            tiles.append((bkt, 0))
    ntiles = len(tiles)
    nblk = ntiles // CB
    ecpad = ntiles * P

    block_specs = []
    for b in range(nblk):
        bts = tiles[b * CB:(b + 1) * CB]
        assert len({t[0] for t in bts}) == 1
        block_specs.append((bts[0][0], tuple(t[1] for t in bts)))

    # slot ranges per (bucket, chunk)
    tile_start = {}
    pos = 0
    for t in tiles:
        tile_start.setdefault(t, []).append(pos)
        pos += P

    in_maps = []
    gathers = []
    iota_col = np.arange(P, dtype=np.float32).reshape(P, 1)
    for c in range(NCORES):
        eids, a0c, l1, bucket, chunk, order = per_core[c]
        i0 = np.zeros(ecpad, np.int32)
        lidx = np.zeros(ecpad, np.int32)
        gid = np.empty(len(eids), np.int64)
        for bkt in range(2):
            for ch in range(SHARD_T):
                sel = order[(bucket[order] == bkt) & (chunk[order] == ch)]
                starts = tile_start[(bkt, ch)]
                for ti in range(len(starts)):
                    seg = sel[ti * P:(ti + 1) * P]
                    dst = starts[ti] + np.arange(len(seg))
                    i0[dst] = a0c[seg] - (HALF if bkt else 0)
                    lidx[dst] = l1[seg] - ch * P
                    gid[seg] = dst
        gathers.append((eids, gid))
        # lidxr[b, j, p] = lidx of edge b*1024 + j*128 + p
        lidxr = lidx.astype(np.float32).reshape(nblk, CB, P)
        in_maps.append({
            "z": zpad[c * SHARD:(c + 1) * SHARD],
            "wcat": wcat,
            "bcat": bcat,
            "wbil": wbil,
            "iotac": iota_col,
            "idx0": _pack_idx(i0.astype(np.int16).reshape(nblk, BLK)),
            "lidxr": lidxr.astype(_np_mdt),
        })

    key = tuple(block_specs)
    if key not in _CACHE:
        _CACHE[key] = _build(block_specs)
    nc = _CACHE[key]

    res = run_bass_kernel_spmd(nc, in_maps, list(range(NCORES)), **_RUN_KWARGS)
    global LAST_RESULTS
    LAST_RESULTS = res

    scores = np.empty(E, np.float32)
    for c in range(NCORES):
        out_c = np.asarray(res.results[c]["out"], np.float32)
        eids, gid = gathers[c]
        scores[eids] = out_c[gid]
    return scores + float(bil_b[0])


def _numpy_reference(z, pot_arcs, lin1_w, lin1_b, lin2_w, lin2_b,
                     bil_w, bil_b, norm_w, norm_b):
    def ln(x):
        mu = x.mean(-1, keepdims=True)
        var = x.var(-1, keepdims=True)
        return (x - mu) / np.sqrt(var + 1e-5) * norm_w + norm_b

    zn = ln(z)
    h1 = ln(np.maximum(zn @ lin1_w.T + lin1_b, 0.0))
    h2 = ln(np.maximum(zn @ lin2_w.T + lin2_b, 0.0))
    g = h1 @ bil_w[0]
    a0 = pot_arcs[:, 0].astype(np.int64)
    a1 = pot_arcs[:, 1].astype(np.int64)
    return np.einsum("ed,ed->e", g[a0], h2[a1]) + bil_b[0]


# revision 21
# speedup vs baseline: 1.0910x; 1.0621x over previous
"""ArcDecoder Bass kernel for 8 TRN2 NeuronCores.

Math (per node n, with norm_w/norm_b folded into weights host-side):
  zn   = LN(z)
  u1   = relu(zn @ W1eff + b1eff),  u2 = relu(zn @ W2eff + b2eff)
  h1n  = LN(u1), h2n = LN(u2)
  g    = h1n @ Wbil_eff
  score_e = dot(g[a0_e], h2n[a1_e]) + bil_b

Phase A (replicated): every core computes the full g/h2 node tables into its
own DRAM (bf16 matmuls, f32 LN stats).
Phase B (edges sharded E/8): per-edge rows fetched with the dma_gather custom
GPSIMD instruction (1024 rows per instruction, round-robin over 4 SWDGE
queues).  dma_gather takes int16 indices, so node ids >= 32768 gather from a
shifted table base; host groups each core's edges into 4 (head-half,
tail-half) buckets so every 1024-edge block is half-pure.  DVE multiply +
reduce forms the dots; host adds bil_b and inverse-permutes.
"""

import sys

if "/opt/trn_rl_repo" not in sys.path:
    sys.path.insert(0, "/opt/trn_rl_repo")

import numpy as np
import ml_dtypes

import concourse.bass as bass
import concourse.tile as tile
from concourse import bacc, mybir
from concourse.bass_utils import run_bass_kernel_spmd
from concourse.library_config import mlp
from concourse.masks import make_identity

N, D, E = 50000, 128, 500000
NCORES = 8
P = 128
SHARD_T = 49                     # node tiles per core (sharded phase A)
SHARD = SHARD_T * P              # 6272 rows per core
NT = SHARD_T * NCORES            # 392 node tiles total
NPAD = NT * P                    # 50176
TB = 4                           # node subtiles per z-load / table-store batch
EC = E // NCORES                 # 62500 edges per core
HALF = 32768                     # int16 index ceiling for dma_gather

F32 = mybir.dt.float32
BF16 = mybir.dt.bfloat16
AF = mybir.ActivationFunctionType
ALU = mybir.AluOpType

TABLE_DT = BF16                  # dtype of g/h2 tables (gather payload)
MM_DT = BF16                     # dtype of matmul operands in phase A
BLK = 1024                       # edges per gather block
CB = BLK // P                    # row-chunks per partition in a gather tile
NQ = 4                           # SWDGE queues
EPS = 1e-5

_np_tdt = np.float32 if TABLE_DT == F32 else ml_dtypes.bfloat16
_np_mdt = np.float32 if MM_DT == F32 else ml_dtypes.bfloat16


def _build(block_specs):
    """block_specs: list of (head_half, tail_half) per 1024-edge block."""
    nblk = len(block_specs)
    ecpad = nblk * BLK
    S = BLK // 16

    nc = bacc.Bacc("TRN2", target_bir_lowering=False, debug=False,
                   num_devices=NCORES, num_swdge_queues=NQ)

    z_ext = nc.dram_tensor("z", [SHARD, D], F32, kind="ExternalInput").ap()
    wcat_ext = nc.dram_tensor("wcat", [D, 2 * D], MM_DT, kind="ExternalInput").ap()
    bcat_ext = nc.dram_tensor("bcat", [1, 2 * D], MM_DT, kind="ExternalInput").ap()
    wbil_ext = nc.dram_tensor("wbil", [D, D], MM_DT, kind="ExternalInput").ap()
    iotac_ext = nc.dram_tensor("iotac", [P, 1], F32, kind="ExternalInput").ap()
    idx0_ext = nc.dram_tensor("idx0", [nblk, P, S], mybir.dt.int16,
                              kind="ExternalInput").ap()
    lidxr_ext = nc.dram_tensor("lidxr", [nblk, 8, P], MM_DT,
                               kind="ExternalInput").ap()
    out_ext = nc.dram_tensor("out", [ecpad], F32, kind="ExternalOutput").ap()

    g_shard = nc.dram_tensor("g_shard", [SHARD, D], TABLE_DT).ap()
    g_table = nc.dram_tensor("g_table", [NPAD, D], TABLE_DT, addr_space="Shared").ap()

    with tile.TileContext(nc) as tc:
        with (
            tc.tile_pool(name="const", bufs=1) as const_p,
            tc.tile_pool(name="zload", bufs=2) as zload_p,
            tc.tile_pool(name="work", bufs=10) as work_p,
            tc.tile_pool(name="stat", bufs=8) as stat_p,
            tc.tile_pool(name="tabout", bufs=2) as tabout_p,
            tc.tile_pool(name="gather", bufs=20) as gather_p,
            tc.tile_pool(name="idx", bufs=20) as idx_p,
            tc.tile_pool(name="score", bufs=4) as score_p,
        ):
            # ---- constants -------------------------------------------------
            nc.gpsimd.load_library(mlp)
            ident = const_p.tile([P, P], MM_DT)
            make_identity(nc, ident[:])
            wcat_sb = const_p.tile([D, 2 * D], MM_DT)
            nc.sync.dma_start(wcat_sb[:], wcat_ext[:])
            bcat_sb = const_p.tile([1, 2 * D], MM_DT)
            nc.sync.dma_start(bcat_sb[:], bcat_ext[:])
            wbil_sb = const_p.tile([D, D], MM_DT)
            nc.sync.dma_start(wbil_sb[:], wbil_ext[:])
            ones_row = const_p.tile([1, P], MM_DT)
            nc.vector.memset(ones_row[:], 1.0)
            epsc = const_p.tile([P, 1], F32)
            nc.vector.memset(epsc[:], EPS)
            h_sb = const_p.tile([P, SHARD_T, D], TABLE_DT)
            iotac_ext2 = iotac_ext  # loaded below
            iotac = const_p.tile([P, 1], F32)
            nc.sync.dma_start(iotac[:], iotac_ext[:])

            # ---- phase A: node tables (sharded; each core its z-shard) ----
            pa = tc.tile_pool(name="psumA", bufs=2, space="PSUM")
            psum_p = pa.__enter__()
            for b0 in range(0, SHARD_T, TB):
                tb = min(TB, SHARD_T - b0)
                n0 = b0 * P
                zbatch = zload_p.tile([P, TB, D], F32, tag="zbatch")
                nc.sync.dma_start(
                    zbatch[:, :tb, :],
                    z_ext[n0:n0 + tb * P, :].rearrange("(j p) d -> p j d", p=P),
                )
                gbuf = tabout_p.tile([P, TB, D], TABLE_DT, tag="gbuf")

                for jj in range(tb):
                    z_t = zbatch[:, jj, :]
                    st = stat_p.tile([P, 6], F32, tag="st")
                    nc.vector.bn_stats(st[:], z_t)
                    mv = stat_p.tile([P, 2], F32, tag="mv")
                    nc.vector.bn_aggr(mv[:], st[:])
                    sd = stat_p.tile([P, 1], F32, tag="sd")
                    nc.scalar.activation(sd[:], mv[:, 1:2], AF.Sqrt, bias=epsc[:])
                    ri = stat_p.tile([P, 1], F32, tag="ri")
                    nc.vector.reciprocal(ri[:], sd[:])
                    zn = work_p.tile([P, D], MM_DT, tag="zn")
                    nc.vector.tensor_scalar(zn[:], z_t, mv[:, 0:1], ri[:],
                                            ALU.subtract, ALU.mult)
                    znT_ps = psum_p.tile([P, P], MM_DT, tag="tpos", space="PSUM")
                    nc.tensor.transpose(znT_ps[:], zn[:], ident[:])
                    znT = work_p.tile([P, P], MM_DT, tag="znT")
                    nc.scalar.copy(znT[:], znT_ps[:])
                    u12_ps = psum_p.tile([P, 2 * D], F32, tag="mm12", space="PSUM")
                    nc.tensor.matmul(u12_ps[:], lhsT=znT[:], rhs=wcat_sb[:],
                                     start=True, stop=False)
                    nc.tensor.matmul(u12_ps[:], lhsT=ones_row[:], rhs=bcat_sb[:],
                                     start=False, stop=True)
                    u12 = work_p.tile([P, 2 * D], F32, tag="u12")
                    nc.scalar.activation(u12[:], u12_ps[:], AF.Relu)
                    st1 = stat_p.tile([P, 6], F32, tag="st1")
                    nc.vector.bn_stats(st1[:], u12[:, 0:D])
                    mv1 = stat_p.tile([P, 2], F32, tag="mv1")
                    nc.vector.bn_aggr(mv1[:], st1[:])
                    sd1 = stat_p.tile([P, 1], F32, tag="sd1")
                    nc.scalar.activation(sd1[:], mv1[:, 1:2], AF.Sqrt, bias=epsc[:])
                    ri1 = stat_p.tile([P, 1], F32, tag="ri1")
                    nc.vector.reciprocal(ri1[:], sd1[:])
                    st2 = stat_p.tile([P, 6], F32, tag="st2")
                    nc.vector.bn_stats(st2[:], u12[:, D:2 * D])
                    mv2 = stat_p.tile([P, 2], F32, tag="mv2")
                    nc.vector.bn_aggr(mv2[:], st2[:])
                    sd2 = stat_p.tile([P, 1], F32, tag="sd2")
                    nc.scalar.activation(sd2[:], mv2[:, 1:2], AF.Sqrt, bias=epsc[:])
                    ri2 = stat_p.tile([P, 1], F32, tag="ri2")
                    nc.vector.reciprocal(ri2[:], sd2[:])
                    h1n = work_p.tile([P, D], MM_DT, tag="h1n")
                    nc.vector.tensor_scalar(h1n[:], u12[:, 0:D], mv1[:, 0:1],
                                            ri1[:], ALU.subtract, ALU.mult)
                    nc.vector.tensor_scalar(h_sb[:, b0 + jj, :], u12[:, D:2 * D],
                                            mv2[:, 0:1], ri2[:],
                                            ALU.subtract, ALU.mult)
                    h1nT_ps = psum_p.tile([P, P], MM_DT, tag="tpos", space="PSUM")
                    nc.tensor.transpose(h1nT_ps[:], h1n[:], ident[:])
                    h1nT = work_p.tile([P, P], MM_DT, tag="h1nT")
                    nc.scalar.copy(h1nT[:], h1nT_ps[:])
                    g_ps = psum_p.tile([P, D], F32, tag="gmm", space="PSUM")
                    nc.tensor.matmul(g_ps[:], lhsT=h1nT[:], rhs=wbil_sb[:],
                                     start=True, stop=True)
                    nc.scalar.copy(gbuf[:, jj, :], g_ps[:])

                nc.sync.dma_start(
                    g_shard[n0:n0 + tb * P, :].rearrange("(j p) d -> p j d", p=P),
                    gbuf[:, :tb, :],
                )


            # ---- all-gather shards into full tables -----------------------
            nc.gpsimd.collective_compute(
                "AllGather", ALU.bypass,
                replica_groups=[list(range(NCORES))],
                ins=[g_shard[:]], outs=[g_table[:]],
            )
            pa.__exit__(None, None, None)
            pb = tc.tile_pool(name="psumB", bufs=4, space="PSUM")
            psum_p = pb.__enter__()


            # ---- phase B: head dma_gather + tail selection matmul ---------
            for b, (h0, chunks) in enumerate(block_specs):
                i0 = idx_p.tile([P, S], mybir.dt.int16, tag="i0")
                nc.sync.dma_start(i0[:], idx0_ext[b])
                lxr = idx_p.tile([1, CB * P], MM_DT, tag="lxr")
                nc.sync.dma_start(lxr[:], lidxr_ext[b].rearrange("j p -> (j p)")[None, :])
                g_src = g_table[HALF:, :] if h0 else g_table[:, :]
                gt = gather_p.tile([P, CB, D], TABLE_DT, tag="gt")
                nc.gpsimd.dma_gather(gt[:], g_src, i0[:], BLK, BLK, D,
                                     queue_num=b % NQ)
                ht = gather_p.tile([P, CB, D], TABLE_DT, tag="ht")
                for j, ch in enumerate(chunks):
                    # broadcast lidx row across partitions via K=1 matmul
                    lb_ps = psum_p.tile([P, P], F32, tag="lbps", space="PSUM")
                    nc.tensor.matmul(lb_ps[:], lhsT=ones_row[:],
                                     rhs=lxr[0:1, j * P:(j + 1) * P], start=True, stop=True)
                    sT = work_p.tile([P, P], MM_DT, tag="sT")
                    nc.vector.tensor_scalar(sT[:], lb_ps[:], iotac[:], None,
                                            ALU.is_equal)
                    sel_ps = psum_p.tile([P, D], F32, tag="selps", space="PSUM")
                    nc.tensor.matmul(sel_ps[:], lhsT=sT[:],
                                     rhs=h_sb[:, ch, :], start=True, stop=True)
                    nc.scalar.copy(ht[:, j, :], sel_ps[:])
                nc.vector.tensor_tensor(gt[:], gt[:], ht[:], op=ALU.mult)
                sc = score_p.tile([P, CB], F32, tag="sc")
                nc.vector.tensor_reduce(
                    sc[:], gt[:], axis=mybir.AxisListType.X, op=ALU.add,
                )
                # edge k of block b sits at [k % 128, k // 128]
                nc.sync.dma_start(
                    out_ext[b * BLK:(b + 1) * BLK].rearrange("(j p) -> p j", p=P),
                    sc[:],
                )

            pb.__exit__(None, None, None)

    nc.compile()
    return nc


_CACHE = {}
_RUN_KWARGS = {}
LAST_RESULTS = None


def _pack_idx(vals):
    """[nblk, 1024] int16 -> dma_gather SBUF layout [nblk, 128, 64]:
    index k lives at partition k%16, column k//16, replicated into all
    eight 16-partition groups."""
    nblk = vals.shape[0]
    w = vals.reshape(nblk, BLK // 16, 16).transpose(0, 2, 1)   # [nblk,16,S]
    return np.tile(w, (1, 8, 1)).astype(np.int16)


def kernel(**inputs) -> np.ndarray:
    z = np.asarray(inputs["z"], np.float32)
    pot_arcs = np.asarray(inputs["pot_arcs"])
    lin1_w = np.asarray(inputs["lin1_w"], np.float32)
    lin1_b = np.asarray(inputs["lin1_b"], np.float32)
    lin2_w = np.asarray(inputs["lin2_w"], np.float32)
    lin2_b = np.asarray(inputs["lin2_b"], np.float32)
    bil_w = np.asarray(inputs["bil_w"], np.float32)
    bil_b = np.asarray(inputs["bil_b"], np.float32)
    norm_w = np.asarray(inputs["norm_w"], np.float32)
    norm_b = np.asarray(inputs["norm_b"], np.float32)

    if not np.allclose(norm_b, 0.0):
        # general norm_b adds per-node scalar terms; not exercised by this
        # problem's inputs.  Exact numpy fallback keeps kernel() total.
        return _numpy_reference(z, pot_arcs, lin1_w, lin1_b, lin2_w, lin2_b,
                                bil_w, bil_b, norm_w, norm_b)

    w1eff = norm_w[:, None] * lin1_w.T
    b1eff = norm_b @ lin1_w.T + lin1_b
    w2eff = norm_w[:, None] * lin2_w.T
    b2eff = norm_b @ lin2_w.T + lin2_b
    wbil = bil_w[0] * norm_w[None, :]
    wcat = np.concatenate([w1eff, w2eff], axis=1).astype(_np_mdt)
    bcat = np.concatenate([b1eff, b2eff])[None, :].astype(_np_mdt)
    wbil = wbil.astype(_np_mdt)

    zpad = np.zeros((NPAD, D), np.float32)
    zpad[:N] = z

    a0 = pot_arcs[:, 0].astype(np.int32)
    a1 = pot_arcs[:, 1].astype(np.int32)

    # --- assign edges to tail-owner cores; tile by (head-half, tail-chunk) -
    core_of_edge = a1 // SHARD
    per_core = []
    for c in range(NCORES):
        eids = np.where(core_of_edge == c)[0]
        a0c = a0[eids]
        l1 = a1[eids] - c * SHARD
        bucket = (a0c >= HALF).astype(np.int64)
        chunk = l1 // P
        # sort by (bucket, chunk) and cut chunk-pure 128-edge tiles
        order = np.lexsort((chunk, bucket))
        per_core.append((eids, a0c, l1, bucket, chunk, order))

    # tiles per (bucket, chunk) must be uniform across cores (same graph)
    ntile_bc = np.zeros((2, SHARD_T), np.int64)
    for c in range(NCORES):
        _, _, _, bucket, chunk, _ = per_core[c]
        for bkt in range(2):
            cnt = np.bincount(chunk[bucket == bkt], minlength=SHARD_T)
            ntile_bc[bkt] = np.maximum(ntile_bc[bkt], -(-cnt // P))
    # tile list: (bucket, chunk) repeated; pad each bucket to multiple of 8
    tiles = []
    for bkt in range(2):
        start = len(tiles)
        for ch in range(SHARD_T):
            tiles += [(bkt, ch)] * int(ntile_bc[bkt, ch])
        while (len(tiles) - start) % CB:
            tiles.append((bkt, 0))
    ntiles = len(tiles)
    nblk = ntiles // CB
    ecpad = ntiles * P

    block_specs = []
    for b in range(nblk):
        bts = tiles[b * CB:(b + 1) * CB]
        assert len({t[0] for t in bts}) == 1
        block_specs.append((bts[0][0], tuple(t[1] for t in bts)))

    # slot ranges per (bucket, chunk)
    tile_start = {}
    pos = 0
    for t in tiles:
        tile_start.setdefault(t, []).append(pos)
        pos += P

    in_maps = []
    gathers = []
    iota_col = np.arange(P, dtype=np.float32).reshape(P, 1)
    for c in range(NCORES):
        eids, a0c, l1, bucket, chunk, order = per_core[c]
        i0 = np.zeros(ecpad, np.int32)
        lidx = np.zeros(ecpad, np.int32)
        gid = np.empty(len(eids), np.int64)
        for bkt in range(2):
            for ch in range(SHARD_T):
                sel = order[(bucket[order] == bkt) & (chunk[order] == ch)]
                starts = tile_start[(bkt, ch)]
                for ti in range(len(starts)):
                    seg = sel[ti * P:(ti + 1) * P]
                    dst = starts[ti] + np.arange(len(seg))
                    i0[dst] = a0c[seg] - (HALF if bkt else 0)
                    lidx[dst] = l1[seg] - ch * P
                    gid[seg] = dst
        gathers.append((eids, gid))
        # lidxr[b, j, p] = lidx of edge b*1024 + j*128 + p
        lidxr = lidx.astype(np.float32).reshape(nblk, CB, P)
        in_maps.append({
            "z": zpad[c * SHARD:(c + 1) * SHARD],
            "wcat": wcat,
            "bcat": bcat,
            "wbil": wbil,
            "iotac": iota_col,
            "idx0": _pack_idx(i0.astype(np.int16).reshape(nblk, BLK)),
            "lidxr": lidxr.astype(_np_mdt),
        })

    key = tuple(block_specs)
    if key not in _CACHE:
        _CACHE[key] = _build(block_specs)
    nc = _CACHE[key]

    res = run_bass_kernel_spmd(nc, in_maps, list(range(NCORES)), **_RUN_KWARGS)
    global LAST_RESULTS
    LAST_RESULTS = res

    scores = np.empty(E, np.float32)
    for c in range(NCORES):
        out_c = np.asarray(res.results[c]["out"], np.float32)
        eids, gid = gathers[c]
        scores[eids] = out_c[gid]
    return scores + float(bil_b[0])


def _numpy_reference(z, pot_arcs, lin1_w, lin1_b, lin2_w, lin2_b,
                     bil_w, bil_b, norm_w, norm_b):
    def ln(x):
        mu = x.mean(-1, keepdims=True)
        var = x.var(-1, keepdims=True)
        return (x - mu) / np.sqrt(var + 1e-5) * norm_w + norm_b

    zn = ln(z)
    h1 = ln(np.maximum(zn @ lin1_w.T + lin1_b, 0.0))
    h2 = ln(np.maximum(zn @ lin2_w.T + lin2_b, 0.0))
    g = h1 @ bil_w[0]
    a0 = pot_arcs[:, 0].astype(np.int64)
    a1 = pot_arcs[:, 1].astype(np.int64)
    return np.einsum("ed,ed->e", g[a0], h2[a1]) + bil_b[0]
